# revision 1
# baseline (speedup 1.0000x reference)
"""Trainium2 Bass kernel for nn_MultiHeadAttention_28260884808093.

MHA without QKV projections: heads formed by reshaping inputs directly,
scores scaled by 1/head_dim (not sqrt), softmax, attn@V, then fc_out.

Sharding: 8 cores = (batch, seq-half). Each core owns a disjoint
[1024, 1024] slice of the final output, so no device collectives are
needed (fc_out mixes head dims, not tokens). Host pre-transposes
q/k/fc_w so every matmul contraction lands on the partition axis.

Matmul operands are bf16 (1 row/cycle on the PE, vs 4 for fp32);
softmax statistics and all accumulation stay fp32.
"""

import os
import sys

sys.path.insert(0, "/opt/trn_rl_repo")

import ml_dtypes
import numpy as np
from contextlib import ExitStack

import concourse.bass as bass  # noqa: F401
import concourse.bacc as bacc
import concourse.tile as tile
from concourse import mybir
from concourse import bass_utils
from concourse.bass_utils import run_bass_kernel_spmd

LDW_OPT = os.environ.get("MHA_LDW_OPT", "0") == "1"
if LDW_OPT and not getattr(bass_utils, "_mha_ldw_patch", False):
    bass_utils._mha_ldw_patch = True
    _orig_run_command = bass_utils.run_command

    def _run_command_ldw(argv, **kwargs):
        if argv and "walrus_driver" in str(argv[0]):
            argv = [
                a.replace("--enable-ldw-opt=false", "--enable-ldw-opt=true")
                for a in argv
            ]
        return _orig_run_command(argv, **kwargs)

    bass_utils.run_command = _run_command_ldw

B, S, D = 4, 2048, 1024
H, DH = 16, 64
N_CORES = 8
M = (B * S) // N_CORES  # 1024 query tokens per core
PAIRS = H // 2  # head pairs (2 heads share a 128-partition tile)
NCH = S // 128  # 16 key chunks of 128
F32 = mybir.dt.float32

# Matmul operand dtype: bfloat16 | float32r | float32
MM_DTYPE = os.environ.get("MHA_MM_DTYPE", "bfloat16")
DT = getattr(mybir.dt, MM_DTYPE)
NP_DT = np.float32 if MM_DTYPE != "bfloat16" else ml_dtypes.bfloat16
WARM_LINKS = int(os.environ.get("MHA_WARM_LINKS", "0"))
WARM_C = int(os.environ.get("MHA_WARM_C", "4"))


def _mha_body(ctx, tc, qT, kT, v, fw, fb, out):
    nc = tc.nc
    sb = ctx.enter_context(tc.tile_pool(name="sb", bufs=1))
    ps = ctx.enter_context(tc.tile_pool(name="ps", bufs=1, space="PSUM"))

    # ---- resident tensors ----
    qt_sb, fw_sb, attn = [], [], []
    for j in range(8):
        qt = sb.tile([128, M], DT, name=f"qt{j}", tag=f"qt{j}")
        nc.sync.dma_start(out=qt[:], in_=qT[j * 128 : (j + 1) * 128, :])
        qt_sb.append(qt)
        fwt = sb.tile([128, D], DT, name=f"fw{j}", tag=f"fw{j}")
        nc.sync.dma_start(out=fwt[:], in_=fw[j * 128 : (j + 1) * 128, :])
        fw_sb.append(fwt)
        at = sb.tile([128, M], DT, name=f"attn{j}", tag=f"attn{j}")
        attn.append(at)

    fb_sb = sb.tile([1, D], F32, name="fb_sb", tag="fb")
    nc.sync.dma_start(out=fb_sb[:], in_=fb[0:1, :])
    fbb = sb.tile([128, D], F32, name="fbb", tag="fbb")
    nc.gpsimd.partition_broadcast(fbb[:], fb_sb[:], channels=128)

    zero_bias = sb.tile([128, 1], F32, name="zero_bias", tag="zb0")
    nc.gpsimd.memset(zero_bias[:], 0.0)

    # PE warm-up gate: the PE HAM clock-gate only flips to 2.4 GHz after
    # an idle-then-burst pattern. Stall pair-0 exps from chunk WARM_C on
    # behind a slow DVE chain so the PE piles up a deep backlog early,
    # then bursts -- flipping HAM warm within the first ~40us instead of
    # after pair 0's sacrificial boundary.
    if WARM_LINKS > 0:
        wprev = sb.tile([1, 4000], F32, name="wc0", tag="wc", bufs=2)
        nc.vector.memset(wprev[:], 0.0)
        for _wi in range(WARM_LINKS):
            wnext = sb.tile([1, 4000], F32, name=f"wc{_wi+1}", tag="wc", bufs=2)
            nc.vector.tensor_copy(wnext[:], wprev[:])
            wprev = wnext
        warm_bias = sb.tile([128, 1], F32, name="warm_bias", tag="wb", bufs=1)
        nc.gpsimd.partition_broadcast(warm_bias[:], wprev[0:1, 0:1], channels=128)
    else:
        warm_bias = zero_bias

    # v as [p, chunk, d] so one DMA per head loads all 16 chunks
    v_pcd = v.rearrange("(c p) d -> p c d", p=128)

    for j in range(PAIRS):
        ha, hb = 2 * j, 2 * j + 1
        kt = sb.tile([128, S], DT, name="kt", tag="kt", bufs=3)
        nc.sync.dma_start(out=kt[:], in_=kT[j * 128 : (j + 1) * 128, :])

        va = sb.tile([128, NCH, DH + 1], DT, name="va", tag="va", bufs=3)
        nc.sync.dma_start(out=va[:, :, 0:DH], in_=v_pcd[:, :, ha * DH : (ha + 1) * DH])
        nc.gpsimd.memset(va[:, :, DH : DH + 1], 1.0)
        vb = sb.tile([128, NCH, DH + 1], DT, name="vb", tag="vb", bufs=3)
        nc.sync.dma_start(out=vb[:, :, 0:DH], in_=v_pcd[:, :, hb * DH : (hb + 1) * DH])
        nc.gpsimd.memset(vb[:, :, DH : DH + 1], 1.0)

        # PV accumulators: rows 0:64 = unnormalized attn_T, row 64 = Z
        oA = ps.tile([DH + 1, M], F32, name="oA", tag="po", bufs=2)
        oB = ps.tile([DH + 1, M], F32, name="oB", tag="po", bufs=2)

        for c in range(NCH):
            sA = ps.tile([128, M], F32, name="sA", tag="st", bufs=2)
            sB = ps.tile([128, M], F32, name="sB", tag="st", bufs=2)
            for s in range(2):
                ms = slice(s * 512, (s + 1) * 512)
                nc.tensor.matmul(
                    sA[:, ms],
                    lhsT=kt[0:64, c * 128 : (c + 1) * 128],
                    rhs=qt_sb[j][0:64, ms],
                    start=True,
                    stop=True,
                    tile_position=(0, 0),
                )
                nc.tensor.matmul(
                    sB[:, ms],
                    lhsT=kt[64:128, c * 128 : (c + 1) * 128],
                    rhs=qt_sb[j][64:128, ms],
                    start=True,
                    stop=True,
                    tile_position=(64, 0),
                )

            ebias = warm_bias if (j == 0 and c >= WARM_C) else zero_bias
            pA = sb.tile([128, M], DT, name="pA", tag="pt", bufs=4)
            nc.scalar.activation(
                out=pA[:], in_=sA[:],
                func=mybir.ActivationFunctionType.Exp,
                bias=ebias[:], scale=1.0 / DH,
            )
            pB = sb.tile([128, M], DT, name="pB", tag="pt", bufs=4)
            nc.scalar.activation(
                out=pB[:], in_=sB[:],
                func=mybir.ActivationFunctionType.Exp,
                bias=ebias[:], scale=1.0 / DH,
            )

            for s in range(2):
                ms = slice(s * 512, (s + 1) * 512)
                nc.tensor.matmul(
                    oA[:, ms], lhsT=va[:, c, :], rhs=pA[:, ms],
                    start=(c == 0), stop=(c == NCH - 1),
                )
                nc.tensor.matmul(
                    oB[:, ms], lhsT=vb[:, c, :], rhs=pB[:, ms],
                    start=(c == 0), stop=(c == NCH - 1),
                )

        # normalize: attn_T[d, m] = oX[d, m] / Z[m].
        # Pair 0 normalizes straight out of PSUM, holding the po slots
        # through the slow reciprocal — the resulting PE idle+backlog is
        # what flips the PE HAM clock-gate to full speed. Later pairs
        # drain PSUM to SBUF immediately (releases po fast, keeps the PE
        # continuously busy and warm) and normalize off the critical path.
        for h, o_ps in ((ha, oA), (hb, oB)):
            po = (h % 2) * 64
            if j == 0:
                rz = sb.tile([1, M], F32, name="rz", tag="rz", bufs=2)
                nc.vector.reciprocal(out=rz[:], in_=o_ps[DH : DH + 1, :])
                zbc = sb.tile([64, M], F32, name="zbc", tag="zbc", bufs=2)
                nc.gpsimd.partition_broadcast(zbc[:], rz[:], channels=64)
                nc.vector.tensor_mul(attn[j][po : po + 64, :], o_ps[0:DH, :], zbc[:])
            else:
                u = sb.tile([DH + 1, M], F32, name="u", tag="un", bufs=3)
                nc.vector.tensor_copy(u[:], o_ps[:])
                rz = sb.tile([1, M], F32, name="rz", tag="rz", bufs=2)
                nc.vector.reciprocal(out=rz[:], in_=u[DH : DH + 1, :])
                zbc = sb.tile([64, M], F32, name="zbc", tag="zbc", bufs=2)
                nc.gpsimd.partition_broadcast(zbc[:], rz[:], channels=64)
                nc.vector.tensor_mul(attn[j][po : po + 64, :], u[0:DH, :], zbc[:])

    # ---- fc_out: out[m, o] = attn_T.T @ fw + b ----
    for mi in range(8):
        for s2 in range(2):
            os_ = slice(s2 * 512, (s2 + 1) * 512)
            pf = ps.tile([128, 512], F32, name="pf", tag="st", bufs=2)
            for j in range(8):
                nc.tensor.matmul(
                    pf[:],
                    lhsT=attn[j][:, mi * 128 : (mi + 1) * 128],
                    rhs=fw_sb[j][:, os_],
                    start=(j == 0),
                    stop=(j == 7),
                )
            ob = sb.tile([128, 512], F32, name="ob", tag="ob", bufs=3)
            nc.vector.tensor_add(ob[:], pf[:], fbb[:, os_])
            nc.sync.dma_start(out=out[mi * 128 : (mi + 1) * 128, os_], in_=ob[:])


def build_module():
    nc = bacc.Bacc("TRN2", target_bir_lowering=False, debug=False, num_devices=N_CORES)
    qT = nc.dram_tensor("qT", [D, M], DT, kind="ExternalInput")
    kT = nc.dram_tensor("kT", [D, S], DT, kind="ExternalInput")
    v = nc.dram_tensor("v", [S, D], DT, kind="ExternalInput")
    fw = nc.dram_tensor("fw", [D, D], DT, kind="ExternalInput")
    fb = nc.dram_tensor("fb", [1, D], F32, kind="ExternalInput")
    out = nc.dram_tensor("out", [M, D], F32, kind="ExternalOutput")
    with tile.TileContext(nc) as tc:
        with ExitStack() as ctx:
            _mha_body(ctx, tc, qT.ap(), kT.ap(), v.ap(), fw.ap(), fb.ap(), out.ap())
    nc.compile()
    return nc


_NC_CACHE = None


def _get_module():
    global _NC_CACHE
    if _NC_CACHE is None:
        _NC_CACHE = build_module()
    return _NC_CACHE


def make_in_maps(query, key, value, fc_w, fc_b):
    fw_host = np.ascontiguousarray(fc_w.T).astype(NP_DT)
    fb_host = np.ascontiguousarray(np.asarray(fc_b, np.float32).reshape(1, D))
    in_maps = []
    kT_cache, v_cache = {}, {}
    for c in range(N_CORES):
        b, half = c // 2, c % 2
        if b not in kT_cache:
            kT_cache[b] = np.ascontiguousarray(key[b].T).astype(NP_DT)
            v_cache[b] = np.ascontiguousarray(value[b]).astype(NP_DT)
        qslice = query[b, half * M : (half + 1) * M, :]
        in_maps.append(
            {
                "qT": np.ascontiguousarray(qslice.T).astype(NP_DT),
                "kT": kT_cache[b],
                "v": v_cache[b],
                "fw": fw_host,
                "fb": fb_host,
            }
        )
    return in_maps


def assemble_out(results):
    out = np.empty((B, S, D), np.float32)
    for c in range(N_CORES):
        b, half = c // 2, c % 2
        out[b, half * M : (half + 1) * M, :] = results[c]["out"]
    return out


def kernel(query, key, value, fc_w, fc_b, _trace=False, _trace_kwargs=None):
    nc = _get_module()
    in_maps = make_in_maps(query, key, value, fc_w, fc_b)
    res = run_bass_kernel_spmd(
        nc,
        in_maps,
        core_ids=list(range(N_CORES)),
        trace=_trace,
        **(_trace_kwargs or {}),
    )
    out = assemble_out(res.results)
    if _trace:
        kernel.last_results = res
    return out


if __name__ == "__main__":
    rng = np.random.default_rng(0)
    q = rng.standard_normal((B, S, D)).astype(np.float32)
    k = rng.standard_normal((B, S, D)).astype(np.float32)
    v = rng.standard_normal((B, S, D)).astype(np.float32)
    w = (rng.standard_normal((D, D)) * 0.03).astype(np.float32)
    bvec = (rng.standard_normal((D,)) * 0.03).astype(np.float32)
    o = kernel(q, k, v, w, bvec)
    print("ran, out shape", o.shape)



# revision 21
# speedup vs baseline: 2.0150x; 2.0150x over previous
"""Trainium2 Bass kernel for nn_MultiHeadAttention_28260884808093.

MHA without QKV projections: heads formed by reshaping inputs directly,
scores scaled by 1/head_dim (not sqrt), softmax, attn@V, then fc_out.

Sharding: 8 cores = (batch, seq-half). Each core owns a disjoint
[1024, 1024] slice of the final output, so no device collectives are
needed (fc_out mixes head dims, not tokens). Host pre-transposes
q/k/fc_w so every matmul contraction lands on the partition axis.

Loop nest: pair j (8) -> m-half mh (2) -> key chunk c (16), flattened
into 256 substeps. Per substep, per engine:
  PE   : QK^T for both heads (row-tiled, concurrent) + the PV
         accumulation lagged PV_LAG substeps (so the PE never waits on
         the exp) -> stays HAM-warm at 2.4 GHz.
  ACT  : exp for the even head (exact, table spline).
  DVE  : exp for the odd head via a Schraudolph bf16 bit-trick
         (one tensor_scalar: i16 = s*128/(ln2*64) + (16256-c), bits
         reinterpreted as bf16). Softmax normalization cancels the
         common-mode error; keeping whole heads pure keeps the residual
         small (measured ~1.0e-2 vs the 2e-2 gate).
  GPSIMD: ONLY partition_broadcast (one library resident -- any second
         gpsimd op type causes ~5us LOAD_LIB swaps per call).
PV accumulators are [65, 512] (1 PSUM bank) per (pair, mh, head), Z
rides as a ones-column in V. Normalize = DVE reciprocal_approx_fast on
the Z row (PSUM src) + gpsimd broadcast + DVE multiply out of PSUM.

The walrus --enable-ldw-opt=true flag (patched below) enables
background weight loads; without it every matmul pays the full
drain+LDWEIGHTS latency (379ns vs 216ns per N=512 bf16 matmul).
"""

import sys

sys.path.insert(0, "/opt/trn_rl_repo")

import ml_dtypes
import numpy as np
from contextlib import ExitStack

import concourse.bass as bass  # noqa: F401
import concourse.bacc as bacc
import concourse.tile as tile
from concourse import mybir
from concourse import bass_utils
from concourse.bass_utils import run_bass_kernel_spmd

B, S, D = 4, 2048, 1024
H, DH = 16, 64
N_CORES = 8
M = (B * S) // N_CORES  # 1024 query tokens per core
PAIRS = H // 2
NCH = S // 128  # 16 key chunks of 128
NSUB = PAIRS * 2 * NCH  # 256 substeps: (pair, m-half, chunk)
PV_LAG = 4  # substeps PV trails QK/exp

F32 = mybir.dt.float32
BF16 = mybir.dt.bfloat16
I16 = mybir.dt.int16
NP_DT = ml_dtypes.bfloat16

# Schraudolph exp->bf16-bits constants (odd heads, DVE):
#   i16 = round(s * 2^7/(ln2*DH) + (127*2^7 - C)) ; bits = bf16
LN2 = float(np.log(2.0))
SCHRAU_C1 = 128.0 / (LN2 * DH)
SCHRAU_C = 8.0
SCHRAU_C2 = 127.0 * 128.0 - SCHRAU_C

# Pairs whose odd head also runs on ACT (engine balance: DVE carries the
# normalize ops, so it takes 7 of the 8 odd heads, not 8).
ACT_B_PAIRS = frozenset({3})


def _sub_idx(u):
    return u // 32, (u // 16) % 2, u % 16  # pair, m-half, chunk


def _mha_body(ctx, tc, qT, kT, v, fw, fb, out):
    nc = tc.nc
    sb = ctx.enter_context(tc.tile_pool(name="sb", bufs=1))
    ps = ctx.enter_context(tc.tile_pool(name="ps", bufs=1, space="PSUM"))

    # ---- resident tensors ----
    qt_sb, fw_sb, attn = [], [], []
    for j in range(8):
        qt = sb.tile([128, M], BF16, name=f"qt{j}", tag=f"qt{j}")
        nc.sync.dma_start(out=qt[:], in_=qT[j * 128 : (j + 1) * 128, :])
        qt_sb.append(qt)
        fwt = sb.tile([128, D], BF16, name=f"fw{j}", tag=f"fw{j}")
        nc.sync.dma_start(out=fwt[:], in_=fw[j * 128 : (j + 1) * 128, :])
        fw_sb.append(fwt)
        at = sb.tile([128, M], BF16, name=f"attn{j}", tag=f"attn{j}")
        attn.append(at)

    fb_sb = sb.tile([1, D], F32, name="fb_sb", tag="fb")
    nc.sync.dma_start(out=fb_sb[:], in_=fb[0:1, :])
    fbb = sb.tile([128, D], F32, name="fbb", tag="fbb")
    nc.gpsimd.partition_broadcast(fbb[:], fb_sb[:], channels=128)

    # v as [p, chunk, d] so one DMA per head loads all 16 chunks
    v_pcd = v.rearrange("(c p) d -> p c d", p=128)

    # K weights zero-padded to the full 128 contraction partitions
    # (ldw-opt rejects row-tiled LDWEIGHTS): head A lives in rows 0:64
    # with zeros below, head B in rows 64:128 with zeros above, so both
    # QK matmuls use full 128x128 weights against the same full-height
    # q tile. Persistent rotating slots -> the zero halves are memset
    # exactly once.
    ktA_slots, ktB_slots = [], []
    for s in range(3):
        ka = sb.tile([128, S], BF16, name=f"ktA{s}", tag=f"ktA{s}")
        nc.vector.memset(ka[64:128, :], 0.0)
        ktA_slots.append(ka)
        kb = sb.tile([128, S], BF16, name=f"ktB{s}", tag=f"ktB{s}")
        nc.vector.memset(kb[0:64, :], 0.0)
        ktB_slots.append(kb)

    kt_t, va_t, vb_t = {}, {}, {}

    def issue_pair_loads(j):
        ka = ktA_slots[j % 3]
        nc.sync.dma_start(out=ka[0:64, :], in_=kT[j * 128 : j * 128 + 64, :])
        kb = ktB_slots[j % 3]
        nc.sync.dma_start(out=kb[64:128, :], in_=kT[j * 128 + 64 : (j + 1) * 128, :])
        kt_t[j] = (ka, kb)
        # V weights at full 128 columns (enables the compiler's automatic
        # fast-weight-load): col 0 = ones (Z lands at PSUM partition 0,
        # 32-aligned for the custom reciprocal), cols 64:128 = v (PV rows
        # at partitions 64:128), cols 1:64 zero.
        va = sb.tile([128, NCH, 128], BF16, name="va", tag="va", bufs=3)
        nc.vector.memset(va[:, :, 0:1], 1.0)
        nc.vector.memset(va[:, :, 1:DH], 0.0)
        nc.sync.dma_start(
            out=va[:, :, DH:128], in_=v_pcd[:, :, (2 * j) * DH : (2 * j + 1) * DH]
        )
        va_t[j] = va
        vb = sb.tile([128, NCH, 128], BF16, name="vb", tag="vb", bufs=3)
        nc.vector.memset(vb[:, :, 0:1], 1.0)
        nc.vector.memset(vb[:, :, 1:DH], 0.0)
        nc.sync.dma_start(
            out=vb[:, :, DH:128], in_=v_pcd[:, :, (2 * j + 1) * DH : (2 * j + 2) * DH]
        )
        vb_t[j] = vb

    issue_pair_loads(0)

    po_tiles = {}
    pa_hist, pb_hist = {}, {}

    def emit_qk_exp(u):
        j, mh, c = _sub_idx(u)
        if mh == 0 and c == 0 and j + 1 < PAIRS:
            issue_pair_loads(j + 1)
        if c == 0:
            oA = ps.tile([128, 512], F32, name="oA", tag="po", bufs=4)
            oB = ps.tile([128, 512], F32, name="oB", tag="po", bufs=4)
            po_tiles[(j, mh)] = (oA, oB)
        ka, kb = kt_t[j]
        ms = slice(mh * 512, (mh + 1) * 512)
        sA = ps.tile([128, 512], F32, name="sA", tag="stA", bufs=2)
        sB = ps.tile([128, 512], F32, name="sB", tag="stB", bufs=2)
        nc.tensor.matmul(
            sA[:],
            lhsT=ka[:, c * 128 : (c + 1) * 128],
            rhs=qt_sb[j][:, ms],
            start=True,
            stop=True,
        )
        nc.tensor.matmul(
            sB[:],
            lhsT=kb[:, c * 128 : (c + 1) * 128],
            rhs=qt_sb[j][:, ms],
            start=True,
            stop=True,
        )
        pa = sb.tile([128, 512], BF16, name="pa", tag="pa", bufs=8)
        nc.scalar.activation(
            out=pa[:],
            in_=sA[:],
            func=mybir.ActivationFunctionType.Exp,
            bias=0.0,
            scale=1.0 / DH,
        )
        if j in ACT_B_PAIRS:
            pb = sb.tile([128, 512], BF16, name="pbx", tag="pb", bufs=8)
            nc.scalar.activation(
                out=pb[:],
                in_=sB[:],
                func=mybir.ActivationFunctionType.Exp,
                bias=0.0,
                scale=1.0 / DH,
            )
        else:
            pb = sb.tile([128, 512], I16, name="pb", tag="pb", bufs=8)
            nc.vector.tensor_scalar(
                out=pb[:],
                in0=sB[:],
                scalar1=SCHRAU_C1,
                scalar2=SCHRAU_C2,
                op0=mybir.AluOpType.mult,
                op1=mybir.AluOpType.add,
            )
        pa_hist[u] = pa
        pb_hist[u] = pb

    def emit_pv(u):
        j, mh, c = _sub_idx(u)
        oA, oB = po_tiles[(j, mh)]
        pa = pa_hist.pop(u)
        pb = pb_hist.pop(u)
        nc.tensor.matmul(
            oA[:],
            lhsT=va_t[j][:, c, :],
            rhs=pa[:],
            start=(c == 0),
            stop=(c == NCH - 1),
        )
        rhs_b = pb[:] if pb.dtype == BF16 else pb[:].bitcast(BF16)
        nc.tensor.matmul(
            oB[:],
            lhsT=vb_t[j][:, c, :],
            rhs=rhs_b,
            start=(c == 0),
            stop=(c == NCH - 1),
        )

    def group_finish_ops(j, mh):
        """Normalize ops for group (j, mh), interleaved one per substep.
        recip + mul on DVE (PSUM source), broadcast on gpsimd."""
        oA, oB = po_tiles.pop((j, mh))
        ms = slice(mh * 512, (mh + 1) * 512)
        ops = []
        for h, oX in ((0, oA), (1, oB)):
            po = h * 64
            rz = sb.tile([1, 512], F32, name="rz", tag="rz", bufs=4)
            zbc = sb.tile([64, 512], F32, name="zbc", tag="zbc", bufs=4)

            def f_recip(oX=oX, rz=rz):
                nc.vector.reciprocal_approx_fast(out=rz[:], in_=oX[0:1, :])

            def f_bcast(zbc=zbc, rz=rz):
                nc.gpsimd.partition_broadcast(zbc[:], rz[:], channels=64)

            def f_mul(oX=oX, zbc=zbc, j=j, po=po, ms=ms):
                nc.vector.tensor_mul(
                    attn[j][po : po + 64, ms], oX[DH:128, :], zbc[:]
                )

            ops += [f_recip, f_bcast, f_mul]
        # release both PSUM slots early: A-ops then B-recip before muls
        return [ops[0], ops[3], ops[1], ops[4], ops[2], ops[5]]

    pending = []
    for u in range(NSUB + PV_LAG):
        if u < NSUB:
            emit_qk_exp(u)
        if u >= PV_LAG:
            up = u - PV_LAG
            emit_pv(up)
            jp, mhp, cp = _sub_idx(up)
            if cp == NCH - 1:
                pending.extend(group_finish_ops(jp, mhp))
        if u < NSUB:
            for f in pending[:2]:
                f()
            del pending[:2]
    for f in pending:
        f()

    # ---- fc_out: out[m, o] = attn_T.T @ fw + b ----
    for mi in range(8):
        for s2 in range(2):
            os_ = slice(s2 * 512, (s2 + 1) * 512)
            pf = ps.tile(
                [128, 512], F32, name="pf", tag=("stA" if s2 == 0 else "stB"), bufs=2
            )
            for jj in range(8):
                nc.tensor.matmul(
                    pf[:],
                    lhsT=attn[jj][:, mi * 128 : (mi + 1) * 128],
                    rhs=fw_sb[jj][:, os_],
                    start=(jj == 0),
                    stop=(jj == 7),
                )
            ob = sb.tile([128, 512], F32, name="ob", tag="ob", bufs=4)
            nc.vector.tensor_add(ob[:], pf[:], fbb[:, os_])
            nc.sync.dma_start(out=out[mi * 128 : (mi + 1) * 128, os_], in_=ob[:])


def build_module():
    nc = bacc.Bacc("TRN2", target_bir_lowering=False, debug=False, num_devices=N_CORES)
    qT = nc.dram_tensor("qT", [D, M], BF16, kind="ExternalInput")
    kT = nc.dram_tensor("kT", [D, S], BF16, kind="ExternalInput")
    v = nc.dram_tensor("v", [S, D], BF16, kind="ExternalInput")
    fw = nc.dram_tensor("fw", [D, D], BF16, kind="ExternalInput")
    fb = nc.dram_tensor("fb", [1, D], F32, kind="ExternalInput")
    out = nc.dram_tensor("out", [M, D], F32, kind="ExternalOutput")
    with tile.TileContext(nc) as tc:
        with ExitStack() as ctx:
            _mha_body(ctx, tc, qT.ap(), kT.ap(), v.ap(), fw.ap(), fb.ap(), out.ap())
    nc.compile()
    return nc


_NC_CACHE = None


def _get_module():
    global _NC_CACHE
    if _NC_CACHE is None:
        _NC_CACHE = build_module()
    return _NC_CACHE


def make_in_maps(query, key, value, fc_w, fc_b):
    fw_host = np.ascontiguousarray(fc_w.T).astype(NP_DT)
    fb_host = np.ascontiguousarray(np.asarray(fc_b, np.float32).reshape(1, D))
    in_maps = []
    kT_cache, v_cache = {}, {}
    for c in range(N_CORES):
        b, half = c // 2, c % 2
        if b not in kT_cache:
            kT_cache[b] = np.ascontiguousarray(key[b].T).astype(NP_DT)
            v_cache[b] = np.ascontiguousarray(value[b]).astype(NP_DT)
        qslice = query[b, half * M : (half + 1) * M, :]
        in_maps.append(
            {
                "qT": np.ascontiguousarray(qslice.T).astype(NP_DT),
                "kT": kT_cache[b],
                "v": v_cache[b],
                "fw": fw_host,
                "fb": fb_host,
            }
        )
    return in_maps


def assemble_out(results):
    out = np.empty((B, S, D), np.float32)
    for c in range(N_CORES):
        b, half = c // 2, c % 2
        out[b, half * M : (half + 1) * M, :] = results[c]["out"]
    return out


def kernel(query, key, value, fc_w, fc_b, _trace=False, _trace_kwargs=None):
    nc = _get_module()
    in_maps = make_in_maps(query, key, value, fc_w, fc_b)
    res = run_bass_kernel_spmd(
        nc,
        in_maps,
        core_ids=list(range(N_CORES)),
        trace=_trace,
        **(_trace_kwargs or {}),
    )
    out = assemble_out(res.results)
    if _trace:
        kernel.last_results = res
    return out


if __name__ == "__main__":
    rng = np.random.default_rng(0)
    q = rng.standard_normal((B, S, D)).astype(np.float32)
    k = rng.standard_normal((B, S, D)).astype(np.float32)
    v = rng.standard_normal((B, S, D)).astype(np.float32)
    w = (rng.standard_normal((D, D)) * 0.03).astype(np.float32)
    bvec = (rng.standard_normal((D,)) * 0.03).astype(np.float32)
    o = kernel(q, k, v, w, bvec)
    print("ran, out shape", o.shape)


# revision 25
# speedup vs baseline: 2.0465x; 1.0156x over previous
"""Trainium2 Bass kernel for nn_MultiHeadAttention_28260884808093.

MHA without QKV projections: heads formed by reshaping inputs directly,
scores scaled by 1/head_dim (not sqrt), softmax, attn@V, then fc_out.

Sharding: 8 cores = (batch, seq-half). Each core owns a disjoint
[1024, 1024] slice of the final output, so no device collectives are
needed (fc_out mixes head dims, not tokens). Host pre-transposes
q/k/fc_w so every matmul contraction lands on the partition axis.

Loop nest: pair j (8) -> m-half mh (2) -> key chunk c (16), flattened
into 256 substeps. Per substep, per engine:
  PE   : QK^T for both heads (row-tiled, concurrent) + the PV
         accumulation lagged PV_LAG substeps (so the PE never waits on
         the exp) -> stays HAM-warm at 2.4 GHz.
  ACT  : exp for the even head (exact, table spline).
  DVE  : exp for the odd head via a Schraudolph bf16 bit-trick
         (one tensor_scalar: i16 = s*128/(ln2*64) + (16256-c), bits
         reinterpreted as bf16). Softmax normalization cancels the
         common-mode error; keeping whole heads pure keeps the residual
         small (measured ~1.0e-2 vs the 2e-2 gate).
  GPSIMD: ONLY partition_broadcast (one library resident -- any second
         gpsimd op type causes ~5us LOAD_LIB swaps per call).
PV accumulators are [65, 512] (1 PSUM bank) per (pair, mh, head), Z
rides as a ones-column in V. Normalize = DVE reciprocal_approx_fast on
the Z row (PSUM src) + gpsimd broadcast + DVE multiply out of PSUM.

All matmul weights are full 128x128 blocks (K zero-padded per head, V
zero-padded across columns): that enables the compiler's automatic
fast-weight-load path, which is what lets back-to-back N=512 bf16
matmuls issue at the 216ns streaming cadence instead of the ~380ns
isolated-matmul latency.
"""

import sys

sys.path.insert(0, "/opt/trn_rl_repo")

import ml_dtypes
import numpy as np
from contextlib import ExitStack

import concourse.bass as bass  # noqa: F401
import concourse.bacc as bacc
import concourse.tile as tile
from concourse import mybir
from concourse import bass_utils
from concourse.bass_utils import run_bass_kernel_spmd

B, S, D = 4, 2048, 1024
H, DH = 16, 64
N_CORES = 8
M = (B * S) // N_CORES  # 1024 query tokens per core
PAIRS = H // 2
NCH = S // 128  # 16 key chunks of 128
NSUB = PAIRS * 2 * NCH  # 256 substeps: (pair, m-half, chunk)
PV_LAG = 4  # substeps PV trails QK/exp

F32 = mybir.dt.float32
BF16 = mybir.dt.bfloat16
I16 = mybir.dt.int16
NP_DT = ml_dtypes.bfloat16

# Schraudolph exp->bf16-bits constants (odd heads, DVE):
#   i16 = round(s * 2^7/(ln2*DH) + (127*2^7 - C)) ; bits = bf16
LN2 = float(np.log(2.0))
SCHRAU_C1 = 128.0 / (LN2 * DH)
SCHRAU_C = 8.0
SCHRAU_C2 = 127.0 * 128.0 - SCHRAU_C

# Pairs whose odd head also runs on ACT (engine balance: DVE carries the
# normalize ops, so it takes 7 of the 8 odd heads, not 8).
ACT_B_PAIRS = frozenset({3})


def _sub_idx(u):
    return u // 32, (u // 16) % 2, u % 16  # pair, m-half, chunk


def _mha_body(ctx, tc, qT, kT, v, fw, fb, out):
    nc = tc.nc
    sb = ctx.enter_context(tc.tile_pool(name="sb", bufs=1))
    ps = ctx.enter_context(tc.tile_pool(name="ps", bufs=1, space="PSUM"))

    # ---- resident tensors (pair-0 critical loads first; fc weights at
    # the end so they don't delay the pipeline ramp) ----
    qt_sb, fw_sb, attn = [], [], []
    for j in range(8):
        qt = sb.tile([128, M], BF16, name=f"qt{j}", tag=f"qt{j}")
        nc.sync.dma_start(out=qt[:], in_=qT[j * 128 : (j + 1) * 128, :])
        qt_sb.append(qt)
        at = sb.tile([128, M], BF16, name=f"attn{j}", tag=f"attn{j}")
        attn.append(at)

    # v as [p, chunk, d] so one DMA per head loads all 16 chunks
    v_pcd = v.rearrange("(c p) d -> p c d", p=128)

    # K weights zero-padded to the full 128 contraction partitions
    # (full-width LDWEIGHTS pipelines; row-tiled ones do not):
    # head A lives in rows 0:64
    # with zeros below, head B in rows 64:128 with zeros above, so both
    # QK matmuls use full 128x128 weights against the same full-height
    # q tile. Persistent rotating slots -> the zero halves are memset
    # exactly once.
    ktA_slots, ktB_slots = [], []
    for s in range(3):
        ka = sb.tile([128, S], BF16, name=f"ktA{s}", tag=f"ktA{s}")
        nc.vector.memset(ka[64:128, :], 0.0)
        ktA_slots.append(ka)
        kb = sb.tile([128, S], BF16, name=f"ktB{s}", tag=f"ktB{s}")
        nc.vector.memset(kb[0:64, :], 0.0)
        ktB_slots.append(kb)

    # V weights at full 128 columns (enables the compiler's automatic
    # fast-weight-load): col 0 = ones (Z lands at PSUM partition 0,
    # 32-aligned for the custom reciprocal), cols 64:128 = v (PV rows
    # at partitions 64:128), cols 1:64 zero. Persistent rotating slots
    # so the constant columns are memset exactly once.
    va_slots, vb_slots = [], []
    for s in range(3):
        for nm, slots in (("va", va_slots), ("vb", vb_slots)):
            vt_ = sb.tile([128, NCH, 128], BF16, name=f"{nm}{s}", tag=f"{nm}{s}")
            nc.vector.memset(vt_[:, :, 0:1], 1.0)
            nc.vector.memset(vt_[:, :, 1:DH], 0.0)
            slots.append(vt_)

    kt_t, va_t, vb_t = {}, {}, {}

    def issue_pair_loads(j):
        ka = ktA_slots[j % 3]
        nc.sync.dma_start(out=ka[0:64, :], in_=kT[j * 128 : j * 128 + 64, :])
        kb = ktB_slots[j % 3]
        nc.sync.dma_start(out=kb[64:128, :], in_=kT[j * 128 + 64 : (j + 1) * 128, :])
        kt_t[j] = (ka, kb)
        va = va_slots[j % 3]
        nc.sync.dma_start(
            out=va[:, :, DH:128], in_=v_pcd[:, :, (2 * j) * DH : (2 * j + 1) * DH]
        )
        va_t[j] = va
        vb = vb_slots[j % 3]
        nc.sync.dma_start(
            out=vb[:, :, DH:128], in_=v_pcd[:, :, (2 * j + 1) * DH : (2 * j + 2) * DH]
        )
        vb_t[j] = vb

    issue_pair_loads(0)

    # fc weights + bias, needed only in the fc phase
    for j in range(8):
        fwt = sb.tile([128, D], BF16, name=f"fw{j}", tag=f"fw{j}")
        nc.sync.dma_start(out=fwt[:], in_=fw[j * 128 : (j + 1) * 128, :])
        fw_sb.append(fwt)
    fb_sb = sb.tile([1, D], F32, name="fb_sb", tag="fb")
    nc.sync.dma_start(out=fb_sb[:], in_=fb[0:1, :])
    fbb = sb.tile([128, D], F32, name="fbb", tag="fbb")
    nc.gpsimd.partition_broadcast(fbb[:], fb_sb[:], channels=128)

    po_tiles = {}
    pa_hist, pb_hist = {}, {}

    def emit_qk_exp(u):
        j, mh, c = _sub_idx(u)
        if mh == 0 and c == 0 and j + 1 < PAIRS:
            issue_pair_loads(j + 1)
        if c == 0:
            oA = ps.tile([128, 512], F32, name="oA", tag="po", bufs=4)
            oB = ps.tile([128, 512], F32, name="oB", tag="po", bufs=4)
            po_tiles[(j, mh)] = (oA, oB)
        ka, kb = kt_t[j]
        ms = slice(mh * 512, (mh + 1) * 512)
        sA = ps.tile([128, 512], F32, name="sA", tag="stA", bufs=2)
        sB = ps.tile([128, 512], F32, name="sB", tag="stB", bufs=2)
        nc.tensor.matmul(
            sA[:],
            lhsT=ka[:, c * 128 : (c + 1) * 128],
            rhs=qt_sb[j][:, ms],
            start=True,
            stop=True,
        )
        nc.tensor.matmul(
            sB[:],
            lhsT=kb[:, c * 128 : (c + 1) * 128],
            rhs=qt_sb[j][:, ms],
            start=True,
            stop=True,
        )
        pa = sb.tile([128, 512], BF16, name="pa", tag="pa", bufs=8)
        nc.scalar.activation(
            out=pa[:],
            in_=sA[:],
            func=mybir.ActivationFunctionType.Exp,
            bias=0.0,
            scale=1.0 / DH,
        )
        if j in ACT_B_PAIRS:
            pb = sb.tile([128, 512], BF16, name="pbx", tag="pb", bufs=8)
            nc.scalar.activation(
                out=pb[:],
                in_=sB[:],
                func=mybir.ActivationFunctionType.Exp,
                bias=0.0,
                scale=1.0 / DH,
            )
        else:
            pb = sb.tile([128, 512], I16, name="pb", tag="pb", bufs=8)
            nc.vector.tensor_scalar(
                out=pb[:],
                in0=sB[:],
                scalar1=SCHRAU_C1,
                scalar2=SCHRAU_C2,
                op0=mybir.AluOpType.mult,
                op1=mybir.AluOpType.add,
            )
        pa_hist[u] = pa
        pb_hist[u] = pb

    def emit_pv(u):
        j, mh, c = _sub_idx(u)
        oA, oB = po_tiles[(j, mh)]
        pa = pa_hist.pop(u)
        pb = pb_hist.pop(u)
        nc.tensor.matmul(
            oA[:],
            lhsT=va_t[j][:, c, :],
            rhs=pa[:],
            start=(c == 0),
            stop=(c == NCH - 1),
        )
        rhs_b = pb[:] if pb.dtype == BF16 else pb[:].bitcast(BF16)
        nc.tensor.matmul(
            oB[:],
            lhsT=vb_t[j][:, c, :],
            rhs=rhs_b,
            start=(c == 0),
            stop=(c == NCH - 1),
        )

    def group_finish_ops(j, mh):
        """Normalize ops for group (j, mh), interleaved one per substep.
        recip + mul on DVE (PSUM source), broadcast on gpsimd."""
        oA, oB = po_tiles.pop((j, mh))
        ms = slice(mh * 512, (mh + 1) * 512)
        ops = []
        for h, oX in ((0, oA), (1, oB)):
            po = h * 64
            rz = sb.tile([1, 512], F32, name="rz", tag="rz", bufs=4)
            zbc = sb.tile([64, 512], F32, name="zbc", tag="zbc", bufs=4)

            def f_recip(oX=oX, rz=rz):
                nc.vector.reciprocal_approx_fast(out=rz[:], in_=oX[0:1, :])

            def f_bcast(zbc=zbc, rz=rz):
                nc.gpsimd.partition_broadcast(zbc[:], rz[:], channels=64)

            def f_mul(oX=oX, zbc=zbc, j=j, po=po, ms=ms):
                nc.vector.tensor_mul(
                    attn[j][po : po + 64, ms], oX[DH:128, :], zbc[:]
                )

            ops += [f_recip, f_bcast, f_mul]
        # release both PSUM slots early: A-ops then B-recip before muls
        return [ops[0], ops[3], ops[1], ops[4], ops[2], ops[5]]

    pending = []
    for u in range(NSUB + PV_LAG):
        if u < NSUB:
            emit_qk_exp(u)
        if u >= PV_LAG:
            up = u - PV_LAG
            emit_pv(up)
            jp, mhp, cp = _sub_idx(up)
            if cp == NCH - 1:
                pending.extend(group_finish_ops(jp, mhp))
        if u < NSUB:
            for f in pending[:2]:
                f()
            del pending[:2]
    for f in pending:
        f()

    # ---- fc_out: out[m, o] = attn_T.T @ fw + b ----
    for mi in range(8):
        for s2 in range(2):
            os_ = slice(s2 * 512, (s2 + 1) * 512)
            pf = ps.tile(
                [128, 512], F32, name="pf", tag=("stA" if s2 == 0 else "stB"), bufs=2
            )
            for jj in range(8):
                nc.tensor.matmul(
                    pf[:],
                    lhsT=attn[jj][:, mi * 128 : (mi + 1) * 128],
                    rhs=fw_sb[jj][:, os_],
                    start=(jj == 0),
                    stop=(jj == 7),
                )
            ob = sb.tile([128, 512], F32, name="ob", tag="ob", bufs=4)
            nc.vector.tensor_add(ob[:], pf[:], fbb[:, os_])
            nc.sync.dma_start(out=out[mi * 128 : (mi + 1) * 128, os_], in_=ob[:])


def build_module():
    nc = bacc.Bacc("TRN2", target_bir_lowering=False, debug=False, num_devices=N_CORES)
    qT = nc.dram_tensor("qT", [D, M], BF16, kind="ExternalInput")
    kT = nc.dram_tensor("kT", [D, S], BF16, kind="ExternalInput")
    v = nc.dram_tensor("v", [S, D], BF16, kind="ExternalInput")
    fw = nc.dram_tensor("fw", [D, D], BF16, kind="ExternalInput")
    fb = nc.dram_tensor("fb", [1, D], F32, kind="ExternalInput")
    out = nc.dram_tensor("out", [M, D], F32, kind="ExternalOutput")
    with tile.TileContext(nc) as tc:
        with ExitStack() as ctx:
            _mha_body(ctx, tc, qT.ap(), kT.ap(), v.ap(), fw.ap(), fb.ap(), out.ap())
    nc.compile()
    return nc


_NC_CACHE = None


def _get_module():
    global _NC_CACHE
    if _NC_CACHE is None:
        _NC_CACHE = build_module()
    return _NC_CACHE


def make_in_maps(query, key, value, fc_w, fc_b):
    fw_host = np.ascontiguousarray(fc_w.T).astype(NP_DT)
    fb_host = np.ascontiguousarray(np.asarray(fc_b, np.float32).reshape(1, D))
    in_maps = []
    kT_cache, v_cache = {}, {}
    for c in range(N_CORES):
        b, half = c // 2, c % 2
        if b not in kT_cache:
            kT_cache[b] = np.ascontiguousarray(key[b].T).astype(NP_DT)
            v_cache[b] = np.ascontiguousarray(value[b]).astype(NP_DT)
        qslice = query[b, half * M : (half + 1) * M, :]
        in_maps.append(
            {
                "qT": np.ascontiguousarray(qslice.T).astype(NP_DT),
                "kT": kT_cache[b],
                "v": v_cache[b],
                "fw": fw_host,
                "fb": fb_host,
            }
        )
    return in_maps


def assemble_out(results):
    out = np.empty((B, S, D), np.float32)
    for c in range(N_CORES):
        b, half = c // 2, c % 2
        out[b, half * M : (half + 1) * M, :] = results[c]["out"]
    return out


def kernel(query, key, value, fc_w, fc_b, _trace=False, _trace_kwargs=None):
    nc = _get_module()
    in_maps = make_in_maps(query, key, value, fc_w, fc_b)
    res = run_bass_kernel_spmd(
        nc,
        in_maps,
        core_ids=list(range(N_CORES)),
        trace=_trace,
        **(_trace_kwargs or {}),
    )
    out = assemble_out(res.results)
    if _trace:
        kernel.last_results = res
    return out


if __name__ == "__main__":
    rng = np.random.default_rng(0)
    q = rng.standard_normal((B, S, D)).astype(np.float32)
    k = rng.standard_normal((B, S, D)).astype(np.float32)
    v = rng.standard_normal((B, S, D)).astype(np.float32)
    w = (rng.standard_normal((D, D)) * 0.03).astype(np.float32)
    bvec = (rng.standard_normal((D,)) * 0.03).astype(np.float32)
    o = kernel(q, k, v, w, bvec)
    print("ran, out shape", o.shape)
